# revision 41
# baseline (speedup 1.0000x reference)
"""BNAF forward + log-det on 8 TRN2 NeuronCores (self-contained).

Sharding: data-parallel over batch (128 rows/core), params replicated.
Host does layout-only prep (transpose / gather / structural masking / constant
fills / dtype storage casts); all arithmetic on input values happens on device.

Math restructuring (validated vs reference in numpy, rel err ~1e-3 bf16):
  masked weight: w = raw-lower-blocks + exp(diag-blocks);  out = (x @ w.T)*s + b
  with s[r] = exp(dw[r] - 0.5*ln(wsn[r])), wsn = row sums of w^2.
  Jacobian log-det chain in linear domain:
    exp(log|dtanh|) = 1 - tanh^2 = derivative_tanh
    E0 = wn0_diag*dtanh(z0); E1 = (blockdiag(wn1)@E0)*s1*dtanh(z1)
    E2 = s2*blockdiag(exp)@E1
    ldj = sum_d [ln(1 + e^gate * E2) - ln(1 + e^gate)] (flows 0,1); sum ln E2 (flow 2)

Perf structure:
  - all matmuls bf16 (f32r at free-dim 128 is quarter-rate on PE), weights
    stored bf16 in DRAM (halves HBM traffic)
  - wsn row-norms via Gram matmuls (chunk.T @ chunk accumulated in PSUM) with
    DVE diagonal extraction (tensor_tensor_reduce against identity) -- no
    elementwise squaring of the big W1 triangle
  - one concatenated DMA for all 16 W1 chunks; all DMAs issued via sync/HWDGE
    (gpsimd dma_start costs ~1us of Pool engine per call in SWDGE)
  - z-scale/bias on DVE from PSUM, coarse-grained Tanh/Derivative_Tanh per
    512-col group (activation-instruction overhead is ~180ns each)
  - activation-table discipline: ln/exp grouped per flow (table load is 1.28us)
"""
import numpy as np
import ml_dtypes

BF = ml_dtypes.bfloat16
DIM, HID, B = 64, 32, 1024
NCORES = 8
BC = B // NCORES
DH = DIM * HID             # 2048
NK = DH // 128             # 16
W1COLS = sum(DH - 128 * k for k in range(NK))   # 17408
W1OFF = [2048 * k - 64 * k * (k - 1) for k in range(NK)]
LOG_2PI = float(np.log(2.0 * np.pi))
NEG = -1e30
MAIN_DT = 'bfloat16'

_CACHE = {}


# ---------------------------------------------------------------- host prep
def _host_prep(inputs):
    fl = {}
    r = np.arange(DH)
    blk = r // HID
    c64 = np.arange(DIM)
    i128 = np.arange(128)
    for f in range(3):
        W0 = np.asarray(inputs[f'W{f}_0'])
        W1 = np.asarray(inputs[f'W{f}_1'])
        W2 = np.asarray(inputs[f'W{f}_2'])

        W0T = np.ascontiguousarray(W0.T)                      # (64, 2048)
        keep = c64[:, None] < blk[None, :]
        diag = c64[:, None] == blk[None, :]
        fl[f'w0raw{f}'] = np.where(keep, W0T, 0.0).astype(BF)
        fl[f'w0exp{f}'] = np.where(diag, W0T, NEG).astype(BF)

        W1T = np.ascontiguousarray(W1.T)                      # (2048, 2048)
        keep1 = blk[:, None] < blk[None, :]                   # all lower 32-blocks
        w1m = np.where(keep1, W1T, 0.0).astype(BF)
        fl[f'w1cat{f}'] = np.concatenate(
            [w1m[128 * k:128 * k + 128, 128 * k:] for k in range(NK)], axis=1)
        d_c = np.empty((32, DH), np.float32)
        for k in range(NK):
            for q in range(4):
                o = 128 * k + 32 * q
                d_c[:, o:o + 32] = W1T[o:o + 32, o:o + 32]
        fl[f'w1dc{f}'] = d_c.astype(BF)

        W2T = np.ascontiguousarray(W2.T)                      # (2048, 64)
        keep2 = c64[None, :] > blk[:, None]
        diag2 = c64[None, :] == blk[:, None]
        w2r = np.where(keep2, W2T, 0.0).astype(np.float32)
        fl[f'w2raw{f}'] = np.ascontiguousarray(
            w2r.reshape(NK, 128, DIM).transpose(1, 0, 2).reshape(128, NK * DIM)
        ).astype(BF)
        w2e = np.where(diag2, W2T, NEG).astype(np.float32)
        fl[f'w2exp{f}'] = np.ascontiguousarray(
            w2e.reshape(NK, 128, DIM).transpose(1, 0, 2).reshape(128, NK * DIM)
        ).astype(BF)

        # packed small params: [dw0|dw1|b0|b1|g0] as (128, 16) column groups
        cols = []
        for nm in (f'dw{f}_0', f'dw{f}_1'):
            cols.append(np.asarray(inputs[nm])[:, 0].reshape(NK, 128).T)
        for nm in (f'b{f}_0', f'b{f}_1'):
            cols.append(np.asarray(inputs[nm]).reshape(NK, 128).T)
        cols.append(W0[np.arange(DH), blk].reshape(NK, 128).T)
        fl[f'sml{f}'] = np.ascontiguousarray(
            np.concatenate(cols, axis=1).astype(np.float32))
        fl[f'sml64{f}'] = np.ascontiguousarray(np.stack(
            [np.asarray(inputs[f'dw{f}_2'])[:, 0],
             np.asarray(inputs[f'b{f}_2'])], axis=1).astype(np.float32))

    fl['gatec0'] = np.full((DIM, 1), float(np.asarray(inputs['gate0'])[0]), np.float32)
    fl['gatec1'] = np.full((DIM, 1), float(np.asarray(inputs['gate1'])[0]), np.float32)
    fl['flip64'] = np.eye(DIM, dtype=np.float32)[:, ::-1].astype(BF)
    fl['ident128'] = np.eye(128, dtype=np.float32)
    fl['ones64'] = np.ones((DIM, 1), np.float32)
    fl['onescol'] = np.ones((128, 1), np.float32)
    return fl


# ------------------------------------------------- walrus sync-wait splitter
def _split_sync_waits(nc, max_waits=1):
    import concourse.mybir as mybir
    for func in nc.m.functions:
        for blkb in func.blocks:
            insts = list(blkb.instructions)
            out = []
            changed = False
            for inst in insts:
                si = inst.sync_info
                if si is not None and len(si.on_wait) > max_waits:
                    waits = list(si.on_wait)
                    keep, pre = waits[-max_waits:], waits[:-max_waits]
                    chunks = [pre[i:i + max_waits] for i in range(0, len(pre), max_waits)]
                    for ci, chunk in enumerate(chunks):
                        nop = mybir.InstNoOp(name=f"{inst.name}.w{ci}", ins=[], outs=[])
                        nop.engine = inst.engine
                        nop.sync_info = mybir.SyncInfo(on_wait=chunk, on_update=[])
                        out.append(nop)
                    inst.sync_info = mybir.SyncInfo(
                        on_wait=keep, on_update=list(si.on_update))
                    changed = True
                out.append(inst)
            if changed:
                try:
                    blkb.instructions = out
                except Exception:
                    while len(blkb.instructions):
                        blkb.remove_instruction(blkb.instructions[-1])
                    for i2 in out:
                        blkb.add_instruction(i2)


# ---------------------------------------------------------------- bass build
def _build(main_dt_name, repeat=1):
    import concourse.bass as bass
    import concourse.mybir as mybir
    import concourse.tile as tile

    f32 = mybir.dt.float32
    bf16 = mybir.dt.bfloat16
    AO = mybir.AluOpType
    AF = mybir.ActivationFunctionType

    nc = bass.Bass()
    P = {}

    def dram(name, shape, dt=f32):
        P[name] = nc.declare_dram_parameter(name, list(shape), dt, isOutput=False)

    dram('xT', (DIM, BC))
    for f in range(3):
        dram(f'w0raw{f}', (DIM, DH), bf16); dram(f'w0exp{f}', (DIM, DH), bf16)
        dram(f'w1cat{f}', (128, W1COLS), bf16)
        dram(f'w1dc{f}', (32, DH), bf16)
        dram(f'w2raw{f}', (128, NK * DIM), bf16)
        dram(f'w2exp{f}', (128, NK * DIM), bf16)
        dram(f'sml{f}', (128, 5 * NK))
        dram(f'sml64{f}', (DIM, 2))
    dram('gatec0', (DIM, 1)); dram('gatec1', (DIM, 1))
    dram('flip64', (DIM, DIM), bf16); dram('ident128', (128, 128))
    dram('ones64', (DIM, 1)); dram('onescol', (128, 1))
    out = nc.declare_dram_parameter('out', [1, BC], f32, isOutput=True)

    with tile.TileContext(nc) as tc:
        with tc.tile_pool(name='const', bufs=1) as cpool, \
             tc.tile_pool(name='wchunk', bufs=2) as wpool, \
             tc.tile_pool(name='acts', bufs=1) as apool, \
             tc.tile_pool(name='sq', bufs=1) as sqpool, \
             tc.tile_pool(name='small', bufs=2) as spool, \
             tc.tile_pool(name='psz', bufs=2, space='PSUM') as pp, \
             tc.tile_pool(name='pse', bufs=2, space='PSUM') as pp1, \
             tc.tile_pool(name='psw', bufs=2, space='PSUM') as ppw:

            def mload(pool, tag, shape, dt, srcap, bufs=None):
                t = pool.tile(list(shape), dt, name=tag, tag=tag, bufs=bufs)
                if dt != srcap.tensor.dtype:
                    nc.gpsimd.dma_start(out=t[:], in_=srcap)
                else:
                    nc.sync.dma_start(out=t[:], in_=srcap)
                return t

            ones64 = mload(cpool, 'ones64', (DIM, 1), f32, P['ones64'][:])
            onescol_bf = mload(cpool, 'onescol_bf', (128, 1), bf16, P['onescol'][:])
            flip64 = mload(cpool, 'flip64', (DIM, DIM), bf16, P['flip64'][:])
            ident = mload(cpool, 'ident', (128, 128), f32, P['ident128'][:])
            xT = mload(cpool, 'xT', (DIM, BC), bf16, P['xT'][:])
            # two persistent expansion buffers for the exp'd W1 32-diag blocks;
            # off-block regions are zeroed once and never written again
            dexpe_bufs = []
            for bi in range(2):
                t = cpool.tile([128, NK, 128], bf16, name=f'dexpe{bi}',
                               tag=f'dexpe{bi}')
                nc.gpsimd.memset(t[:], 0.0)
                dexpe_bufs.append(t)
            prep_count = [0]

            for _rep in range(repeat):
              if _rep:
                xT = mload(cpool, 'xT', (DIM, BC), bf16, P['xT'][:])
              acc = cpool.tile([DIM, BC], f32, name='acc', tag='acc')
              nc.vector.memset(acc[:], 0.0)
              E2s = [cpool.tile([DIM, BC], f32, name=f'E2_{f}', tag=f'E2_{f}') for f in range(3)]
              egs = [cpool.tile([DIM, 1], f32, name=f'eg_{f}', tag=f'eg_{f}') for f in range(2)]
              sgs = []

              # gate scalars for both flows up-front (exp+tanh co-resident)
              for f in range(2):
                    gc = mload(spool, f'gc{f}', (DIM, 1), f32, P[f'gatec{f}'][:])
                    th = spool.tile([DIM, 1], f32, name=f'th{f}', tag=f'th{f}')
                    nc.scalar.activation(th[:], gc[:], AF.Tanh, scale=0.5)
                    sg = spool.tile([DIM, 1], f32, name=f'sg{f}', tag=f'sg{f}')
                    nc.vector.tensor_scalar(out=sg[:], in0=th[:], scalar1=0.5,
                                            scalar2=0.5, op0=AO.mult, op1=AO.add)
                    omsg = spool.tile([DIM, 1], f32, name=f'omsg{f}', tag=f'omsg{f}')
                    nc.vector.tensor_scalar(out=omsg[:], in0=sg[:], scalar1=-1.0,
                                            scalar2=1.0, op0=AO.mult, op1=AO.add)
                    nc.scalar.activation(egs[f][:], gc[:], AF.Exp)
                    sgs.append((sg, omsg))

              def prep(f):
                    # loads (all via sync/HWDGE) + device exps; double-buffered
                    # so flow f+1's prep overlaps flow f's compute
                    w0raw = mload(apool, 'w0raw', (DIM, DH), bf16,
                                  P[f'w0raw{f}'][:], bufs=2)
                    w0exp_in = mload(apool, 'w0expin', (DIM, DH), bf16,
                                     P[f'w0exp{f}'][:], bufs=2)
                    w1cat = mload(wpool, 'w1cat', (128, W1COLS), bf16,
                                  P[f'w1cat{f}'][:])
                    w1dc_in = mload(apool, 'w1dcin', (32, NK, 128), bf16,
                                    P[f'w1dc{f}'][:], bufs=2)
                    w2raw = mload(apool, 'w2raw', (128, NK * DIM), bf16,
                                  P[f'w2raw{f}'][:], bufs=2)
                    w2exp_in = mload(apool, 'w2expin', (128, NK * DIM), bf16,
                                     P[f'w2exp{f}'][:], bufs=2)
                    sml = mload(spool, 'sml', (128, 5 * NK), f32, P[f'sml{f}'][:])
                    sml64 = mload(spool, 'sml64', (DIM, 2), f32, P[f'sml64{f}'][:])

                    w0expt = apool.tile([DIM, DH], bf16, name='w0expt',
                                        tag='w0expt', bufs=2)
                    nc.scalar.activation(w0expt[:], w0exp_in[:], AF.Exp)
                    w0all = apool.tile([DIM, DH], bf16, name='w0all',
                                       tag='w0all', bufs=2)
                    nc.gpsimd.tensor_tensor(w0all[:], w0raw[:], w0expt[:], AO.add)
                    ce3 = apool.tile([32, NK, 128], bf16, name='ce3',
                                     tag='ce3', bufs=2)
                    nc.scalar.activation(ce3[:], w1dc_in[:], AF.Exp)
                    # expand compact exp'd 32-diag blocks into a pre-zeroed
                    # persistent tile via 4 strided SBUF->SBUF DMAs
                    dexpe = dexpe_bufs[prep_count[0] % 2]
                    prep_count[0] += 1
                    for q in range(4):
                        nc.sync.dma_start(
                            out=dexpe[32 * q:32 * q + 32, :, 32 * q:32 * q + 32],
                            in_=ce3[:, :, 32 * q:32 * q + 32])
                    # fold exp'd 32-diag blocks into w1cat's in-chunk columns
                    # (those slots hold only the strictly-lower 32-blocks; the
                    # 32-diag positions are structurally zero, so add == insert)
                    for k in range(NK):
                        eng = nc.gpsimd if k % 2 else nc.vector
                        eng.tensor_tensor(
                            w1cat[:, W1OFF[k]:W1OFF[k] + 128],
                            w1cat[:, W1OFF[k]:W1OFF[k] + 128],
                            dexpe[:, k:k + 1, :], AO.add)
                    w2e = apool.tile([128, NK * DIM], bf16, name='w2e',
                                     tag='w2e', bufs=2)
                    nc.scalar.activation(w2e[:], w2exp_in[:], AF.Exp)
                    w2all = apool.tile([128, NK * DIM], bf16, name='w2all',
                                       tag='w2all', bufs=2)
                    nc.gpsimd.tensor_tensor(w2all[:], w2raw[:], w2e[:], AO.add)
                    eg0 = spool.tile([128, NK], f32, name='eg0', tag='eg0')
                    nc.scalar.activation(eg0[:], sml[:, 4 * NK:5 * NK], AF.Exp)
                    return dict(w0all=w0all, w1cat=w1cat, dexpe=dexpe,
                                w2all=w2all, w2e=w2e, eg0=eg0,
                                dw0=sml[:, 0:NK], dw1=sml[:, NK:2 * NK],
                                b0=sml[:, 2 * NK:3 * NK], b1=sml[:, 3 * NK:4 * NK],
                                dw2=sml64[:, 0:1], b2=sml64[:, 1:2])

              pd = prep(0)
              for f in range(3):
                    w0all, w1cat, dexpe = pd['w0all'], pd['w1cat'], pd['dexpe']
                    w2all, w2e, eg0 = pd['w2all'], pd['w2e'], pd['eg0']
                    dw0, dw1, b0, b1 = pd['dw0'], pd['dw1'], pd['b0'], pd['b1']
                    dw2, b2 = pd['dw2'], pd['b2']

                    # ---- wsn via Gram matmuls + diagonal extraction
                    # (mask diag with identity on DVE; column sums of the
                    # masked Gram equal its diagonal, so one N=1 matmul against
                    # a ones column drops the diagonal into the wsn psum)
                    wsnp = ppw.tile([128, 33], f32, name='wsnp', tag='wsnp',
                                    bufs=1)

                    def diag_extract(G, n, wsncol):
                        gm = sqpool.tile([128, 128], bf16, name='gscr', tag='gscr',
                                         bufs=2)
                        nc.vector.tensor_tensor(gm[:n, :n], G[:], ident[:n, :n],
                                                AO.mult)
                        nc.tensor.matmul(wsncol, gm[:n, :n], onescol_bf[:n, :],
                                         start=True, stop=True)

                    for m in range(NK):
                        G = ppw.tile([128, 128], f32, name='g1', tag='g1')
                        sl = w0all[:, 128 * m:128 * m + 128]
                        nc.tensor.matmul(G[:], sl, sl, start=True, stop=True)
                        diag_extract(G, 128, wsnp[:, m:m + 1])

                    for m in range(NK - 1, -1, -1):
                        G = ppw.tile([128, 128], f32, name='g1', tag='g1')
                        for k in range(m + 1):
                            sl = w1cat[:, W1OFF[k] + 128 * (m - k):
                                       W1OFF[k] + 128 * (m - k) + 128]
                            nc.tensor.matmul(G[:], sl, sl, start=(k == 0),
                                             stop=(k == m))
                        diag_extract(G, 128, wsnp[:, NK + m:NK + m + 1])

                    # ---- wsn2 via Gram
                    G2 = ppw.tile([DIM, DIM], f32, name='g2', tag='g1')
                    for k in range(NK):
                        sl = w2all[:, k * DIM:(k + 1) * DIM]
                        nc.tensor.matmul(G2[:], sl, sl, start=(k == 0), stop=(k == NK - 1))
                    diag_extract(G2, DIM, wsnp[:DIM, 32:33])

                    # ---- s-phase (ln+exp grouped: 2 table swaps per flow)
                    s0 = _scol(nc, spool, AO, AF, wsnp[:, 0:NK], dw0, (128, NK), 's0')
                    s1 = _scol(nc, spool, AO, AF, wsnp[:, NK:2 * NK], dw1, (128, NK), 's1')
                    s2 = _scol(nc, spool, AO, AF, wsnp[:DIM, 32:33], dw2, (DIM, 1), 's2')

                    c0 = spool.tile([128, NK], f32, name='c0', tag='c0')
                    nc.vector.tensor_tensor(c0[:], eg0[:], s0[:], AO.mult)
                    c0n = spool.tile([128, NK], f32, name='c0n', tag='c0n')
                    nc.vector.tensor_scalar(out=c0n[:], in0=c0[:], scalar1=-1.0,
                                            scalar2=None, op0=AO.mult)
                    s1n = spool.tile([128, NK], f32, name='s1n', tag='s1n')
                    nc.vector.tensor_scalar(out=s1n[:], in0=s1[:], scalar1=-1.0,
                                            scalar2=None, op0=AO.mult)

                    # ---- layer 0: matmul -> DVE affine -> coarse tanh + square
                    zs0 = apool.tile([128, DH], f32, name='zs0', tag='zs0')
                    h0 = apool.tile([128, DH], bf16, name='h0', tag='h0')
                    hq0 = apool.tile([128, DH], bf16, name='hq0', tag='hq0')
                    E0 = apool.tile([128, DH], bf16, name='E0', tag='E0')
                    for g in range(4):
                        ps = pp.tile([128, 512], f32, name='zmain', tag='zmain')
                        for j in range(4):
                            m = 4 * g + j
                            nc.tensor.matmul(ps[:, 128 * j:128 * j + 128],
                                             w0all[:, 128 * m:128 * m + 128], xT[:],
                                             start=True, stop=True)
                        for j in range(4):
                            m = 4 * g + j
                            nc.vector.tensor_scalar(
                                out=zs0[:, 128 * m:128 * m + 128],
                                in0=ps[:, 128 * j:128 * j + 128],
                                scalar1=s0[:, m:m + 1], scalar2=b0[:, m:m + 1],
                                op0=AO.mult, op1=AO.add)
                        gsl = slice(512 * g, 512 * g + 512)
                        nc.scalar.activation(h0[:, gsl], zs0[:, gsl], AF.Tanh)
                        nc.scalar.square(hq0[:, gsl], h0[:, gsl])
                        for j in range(4):
                            m = 4 * g + j
                            # E0 = c0 * (1 - h0^2)
                            nc.vector.tensor_scalar(
                                out=E0[:, 128 * m:128 * m + 128],
                                in0=hq0[:, 128 * m:128 * m + 128],
                                scalar1=c0n[:, m:m + 1], scalar2=c0[:, m:m + 1],
                                op0=AO.mult, op1=AO.add)

                    # ---- layer 1 (+ E-path combine)
                    zs1 = apool.tile([128, DH], f32, name='zs1', tag='zs1')
                    h1 = apool.tile([128, DH], bf16, name='h1', tag='h1')
                    hq1 = apool.tile([128, DH], bf16, name='hq1', tag='hq1')
                    ets = apool.tile([128, DH], bf16, name='ets', tag='ets')
                    E1 = apool.tile([128, DH], bf16, name='E1', tag='E1')
                    for g in range(4):
                        ps = pp.tile([128, 512], f32, name='zmain', tag='zmain')
                        for j in range(4):
                            m = 4 * g + j
                            sl = ps[:, 128 * j:128 * j + 128]
                            for k in range(m + 1):
                                nc.tensor.matmul(
                                    sl, w1cat[:, W1OFF[k] + 128 * (m - k):
                                              W1OFF[k] + 128 * (m - k) + 128],
                                    h0[:, 128 * k:128 * k + 128],
                                    start=(k == 0), stop=(k == m))
                        psE = pp1.tile([128, 512], f32, name='epath', tag='epath')
                        for j in range(4):
                            m = 4 * g + j
                            nc.tensor.matmul(psE[:, 128 * j:128 * j + 128],
                                             dexpe[:, m:m + 1, :],
                                             E0[:, 128 * m:128 * m + 128],
                                             start=True, stop=True)
                        for j in range(4):
                            m = 4 * g + j
                            nc.vector.tensor_scalar(
                                out=zs1[:, 128 * m:128 * m + 128],
                                in0=ps[:, 128 * j:128 * j + 128],
                                scalar1=s1[:, m:m + 1], scalar2=b1[:, m:m + 1],
                                op0=AO.mult, op1=AO.add)
                        gsl = slice(512 * g, 512 * g + 512)
                        nc.scalar.activation(h1[:, gsl], zs1[:, gsl], AF.Tanh)
                        nc.scalar.square(hq1[:, gsl], h1[:, gsl])
                        for j in range(4):
                            m = 4 * g + j
                            # ets = s1 * (1 - h1^2)
                            nc.vector.tensor_scalar(
                                out=ets[:, 128 * m:128 * m + 128],
                                in0=hq1[:, 128 * m:128 * m + 128],
                                scalar1=s1n[:, m:m + 1], scalar2=s1[:, m:m + 1],
                                op0=AO.mult, op1=AO.add)
                        nc.vector.tensor_tensor(E1[:, gsl], psE[:], ets[:, gsl],
                                                AO.mult)

                    # ---- prefetch next flow behind this flow's tail
                    if f < 2:
                        pd = prep(f + 1)

                    # ---- layer 2
                    psz2 = pp.tile([DIM, BC], f32, name='zsmall', tag='zsmall',
                                   bufs=1)
                    for k in range(NK):
                        nc.tensor.matmul(psz2[:], w2all[:, k * DIM:(k + 1) * DIM],
                                         h1[:, 128 * k:128 * k + 128],
                                         start=(k == 0), stop=(k == NK - 1))
                    z2 = spool.tile([DIM, BC], f32, name='z2s', tag='z2s')
                    nc.vector.tensor_scalar(out=z2[:], in0=psz2[:],
                                            scalar1=s2[:, 0:1], scalar2=b2,
                                            op0=AO.mult, op1=AO.add)
                    psE2 = pp1.tile([DIM, BC], f32, name='e2small', tag='epath')
                    for k in range(NK):
                        nc.tensor.matmul(psE2[:], w2e[:, k * DIM:(k + 1) * DIM],
                                         E1[:, 128 * k:128 * k + 128],
                                         start=(k == 0), stop=(k == NK - 1))
                    nc.vector.tensor_scalar(out=E2s[f][:], in0=psE2[:],
                                            scalar1=s2[:, 0:1], scalar2=None,
                                            op0=AO.mult)

                    # ---- gate mix / flip or final logp term
                    if f < 2:
                        sg, omsg = sgs[f]
                        ta = spool.tile([DIM, BC], f32, name='ta', tag='ta')
                        nc.gpsimd.tensor_scalar(out=ta[:], in0=z2[:], scalar1=sg[:],
                                                scalar2=None, op0=AO.mult)
                        tb = spool.tile([DIM, BC], f32, name='tb', tag='tb')
                        nc.gpsimd.tensor_scalar(out=tb[:], in0=xT[:], scalar1=omsg[:],
                                                scalar2=None, op0=AO.mult)
                        xmix = spool.tile([DIM, BC], bf16, name='xmix', tag='xmix')
                        nc.gpsimd.tensor_tensor(xmix[:], ta[:], tb[:], AO.add)
                        psf = pp.tile([DIM, BC], f32, name='zsmall', tag='zsmall',
                                      bufs=1)
                        nc.tensor.matmul(psf[:], flip64[:], xmix[:], start=True, stop=True)
                        nc.vector.tensor_copy(xT[:], psf[:])
                    else:
                        sqx = spool.tile([DIM, BC], f32, name='sqx', tag='sqx')
                        nc.gpsimd.tensor_tensor(sqx[:], z2[:], z2[:], AO.mult)
                        nc.gpsimd.tensor_scalar(out=sqx[:], in0=sqx[:], scalar1=-0.5,
                                                scalar2=-0.5 * LOG_2PI, op0=AO.mult, op1=AO.add)
                        nc.gpsimd.tensor_tensor(acc[:], acc[:], sqx[:], AO.add)

              # ---- ldj tail (ln phase)
              for f in range(2):
                    u = spool.tile([DIM, BC], f32, name='u', tag='u')
                    nc.gpsimd.tensor_scalar(out=u[:], in0=E2s[f][:], scalar1=egs[f][:],
                                            scalar2=1.0, op0=AO.mult, op1=AO.add)
                    lf = spool.tile([DIM, BC], f32, name='lf', tag='lf')
                    nc.scalar.activation(lf[:], u[:], AF.Ln)
                    l1p = spool.tile([DIM, 1], f32, name='l1p', tag='l1p')
                    nc.vector.tensor_scalar(out=l1p[:], in0=egs[f][:], scalar1=1.0,
                                            scalar2=None, op0=AO.add)
                    nc.scalar.activation(l1p[:], l1p[:], AF.Ln)
                    nc.gpsimd.tensor_scalar(out=lf[:], in0=lf[:], scalar1=l1p[:],
                                            scalar2=None, op0=AO.subtract)
                    nc.gpsimd.tensor_tensor(acc[:], acc[:], lf[:], AO.add)
              lf2 = spool.tile([DIM, BC], f32, name='lf2', tag='lf2')
              nc.scalar.activation(lf2[:], E2s[2][:], AF.Ln)
              nc.gpsimd.tensor_tensor(acc[:], acc[:], lf2[:], AO.add)

              psum_out = pp.tile([1, BC], f32, name='zsmall', tag='zsmall',
                                 bufs=1)
              nc.tensor.matmul(psum_out[:], ones64[:], acc[:], start=True, stop=True)
              outs = spool.tile([1, BC], f32, name='outs', tag='outs')
              nc.vector.tensor_copy(outs[:], psum_out[:])
              nc.sync.dma_start(out=out[:], in_=outs[:])

    _split_sync_waits(nc)
    return nc


def _scol(nc, spool, AO, AF, wsn, dwc, shape, tag):
    """s = exp(dw - 0.5*ln(wsn)) as (P, ncols) tile."""
    import concourse.mybir as mybir
    f32 = mybir.dt.float32
    ln = spool.tile(list(shape), f32, name=tag + 'n' + 'ln')
    nc.scalar.activation(ln[:], wsn[:], AF.Ln)
    t = spool.tile(list(shape), f32, name=tag + 'n' + 't')
    nc.vector.tensor_scalar(out=t[:], in0=ln[:], scalar1=-0.5, scalar2=None, op0=AO.mult)
    nc.vector.tensor_tensor(t[:], t[:], dwc, AO.add)
    s = spool.tile(list(shape), f32, name=tag + 'n' + 's')
    nc.scalar.activation(s[:], t[:], AF.Exp)
    return s


# ------------------------------------------------------------------ runner
def _make_runner(nc, n_cores):
    import jax
    from jax.sharding import Mesh, PartitionSpec
    from jax.experimental.shard_map import shard_map
    import concourse.mybir as mybir
    from concourse.bass2jax import (_bass_exec_p, partition_id_tensor,
                                    install_neuronx_cc_hook)
    install_neuronx_cc_hook()
    partition_name = nc.partition_id_tensor.name if nc.partition_id_tensor else None
    in_names, out_names, out_avals = [], [], []
    for alloc in nc.m.functions[0].allocations:
        if not isinstance(alloc, mybir.MemoryLocationSet):
            continue
        name = alloc.memorylocations[0].name
        if alloc.kind == "ExternalInput":
            if name != partition_name:
                in_names.append(name)
        elif alloc.kind == "ExternalOutput":
            out_names.append(name)
            out_avals.append(jax.core.ShapedArray(
                tuple(alloc.tensor_shape), mybir.dt.np(alloc.dtype)))
    n_params = len(in_names)
    all_names = in_names + out_names + ([partition_name] if partition_name else [])

    def _body(*args):
        operands = list(args)
        if partition_name is not None:
            operands.append(partition_id_tensor())
        outs = _bass_exec_p.bind(
            *operands, out_avals=tuple(out_avals), in_names=tuple(all_names),
            out_names=tuple(out_names), lowering_input_output_aliases=(),
            sim_require_finite=False, sim_require_nnan=False, nc=nc)
        return tuple(outs)

    devices = jax.devices()[:n_cores]
    mesh = Mesh(np.asarray(devices), ("core",))
    n_outs = len(out_names)
    in_specs = (PartitionSpec("core"),) * (n_params + n_outs)
    out_specs = (PartitionSpec("core"),) * n_outs
    fn = jax.jit(shard_map(_body, mesh=mesh, in_specs=in_specs,
                           out_specs=out_specs, check_rep=False),
                 keep_unused=True)
    return fn, in_names, out_names, out_avals


def _get_runner():
    key = ('runner', MAIN_DT)
    if key not in _CACHE:
        import sys, os
        d = os.path.dirname(os.path.abspath(__file__))
        if d not in sys.path:
            sys.path.insert(0, d)
        nc = _build(MAIN_DT)
        _CACHE[key] = _make_runner(nc, NCORES)
    return _CACHE[key]


def kernel(**inputs):
    fl = _host_prep(inputs)
    x = np.asarray(inputs['x'])
    fn, in_names, out_names, out_avals = _get_runner()
    in_maps = []
    for c in range(NCORES):
        m = dict(fl)
        m['xT'] = np.ascontiguousarray(x[c * BC:(c + 1) * BC, :].T)
        in_maps.append(m)
    concat_in = [np.concatenate([np.asarray(m[name]) for m in in_maps], axis=0)
                 for name in in_names]
    concat_zeros = [np.zeros((NCORES * a.shape[0], *a.shape[1:]), a.dtype)
                    for a in out_avals]
    outs = fn(*concat_in, *concat_zeros)
    o = np.asarray(outs[0]).reshape(NCORES, BC)
    return o.reshape(B).astype(np.float32)


# revision 43
# speedup vs baseline: 1.4862x; 1.4862x over previous
"""BNAF forward + log-det on 8 TRN2 NeuronCores (self-contained).

Sharding: data-parallel over batch (128 rows/core), params replicated.
Host does layout-only prep (transpose / gather / structural masking / constant
fills / dtype storage casts); all arithmetic on input values happens on device.

Math restructuring (validated vs reference in numpy, rel err ~1e-3 bf16):
  masked weight: w = raw-lower-blocks + exp(diag-blocks);  out = (x @ w.T)*s + b
  with s[r] = exp(dw[r] - 0.5*ln(wsn[r])), wsn = row sums of w^2.
  Jacobian log-det chain in linear domain:
    exp(log|dtanh|) = 1 - tanh^2 = derivative_tanh
    E0 = wn0_diag*dtanh(z0); E1 = (blockdiag(wn1)@E0)*s1*dtanh(z1)
    E2 = s2*blockdiag(exp)@E1
    ldj = sum_d [ln(1 + e^gate * E2) - ln(1 + e^gate)] (flows 0,1); sum ln E2 (flow 2)

Perf structure:
  - all matmuls bf16 (f32r at free-dim 128 is quarter-rate on PE), weights
    stored bf16 in DRAM (halves HBM traffic)
  - wsn row-norms via Gram matmuls (chunk.T @ chunk accumulated in PSUM) with
    DVE diagonal extraction (tensor_tensor_reduce against identity) -- no
    elementwise squaring of the big W1 triangle
  - one concatenated DMA for all 16 W1 chunks; all DMAs issued via sync/HWDGE
    (gpsimd dma_start costs ~1us of Pool engine per call in SWDGE)
  - z-scale/bias on DVE from PSUM, coarse-grained Tanh/Derivative_Tanh per
    512-col group (activation-instruction overhead is ~180ns each)
  - activation-table discipline: ln/exp grouped per flow (table load is 1.28us)
"""
import numpy as np
import ml_dtypes

BF = ml_dtypes.bfloat16
DIM, HID, B = 64, 32, 1024
NCORES = 8
BC = B // NCORES
DH = DIM * HID             # 2048
NK = DH // 128             # 16
W1COLS = sum(DH - 128 * k for k in range(NK))   # 17408
W1OFF = [2048 * k - 64 * k * (k - 1) for k in range(NK)]
LOG_2PI = float(np.log(2.0 * np.pi))
NEG = -1e30
MAIN_DT = 'bfloat16'

_CACHE = {}


# ---------------------------------------------------------------- host prep
def _host_prep(inputs):
    fl = {}
    r = np.arange(DH)
    blk = r // HID
    c64 = np.arange(DIM)
    i128 = np.arange(128)
    for f in range(3):
        W0 = np.asarray(inputs[f'W{f}_0'])
        W1 = np.asarray(inputs[f'W{f}_1'])
        W2 = np.asarray(inputs[f'W{f}_2'])

        W0T = np.ascontiguousarray(W0.T)                      # (64, 2048)
        keep = c64[:, None] < blk[None, :]
        diag = c64[:, None] == blk[None, :]
        fl[f'w0raw{f}'] = np.where(keep, W0T, 0.0).astype(BF)
        fl[f'w0exp{f}'] = np.where(diag, W0T, NEG).astype(BF)

        W1T = np.ascontiguousarray(W1.T)                      # (2048, 2048)
        keep1 = blk[:, None] < blk[None, :]                   # all lower 32-blocks
        w1m = np.where(keep1, W1T, 0.0).astype(BF)
        fl[f'w1cat{f}'] = np.concatenate(
            [w1m[128 * k:128 * k + 128, 128 * k:] for k in range(NK)], axis=1)
        d_exp = np.full((128, DH), NEG, np.float32)
        dia = (i128[:, None] // HID) == (i128[None, :] // HID)
        for k in range(NK):
            t = W1T[128 * k:128 * k + 128, 128 * k:128 * k + 128]
            d_exp[:, 128 * k:128 * k + 128] = np.where(dia, t, NEG)
        fl[f'w1dexp{f}'] = d_exp.astype(BF)

        W2T = np.ascontiguousarray(W2.T)                      # (2048, 64)
        keep2 = c64[None, :] > blk[:, None]
        diag2 = c64[None, :] == blk[:, None]
        w2r = np.where(keep2, W2T, 0.0).astype(np.float32)
        fl[f'w2raw{f}'] = np.ascontiguousarray(
            w2r.reshape(NK, 128, DIM).transpose(1, 0, 2).reshape(128, NK * DIM)
        ).astype(BF)
        w2e = np.where(diag2, W2T, NEG).astype(np.float32)
        fl[f'w2exp{f}'] = np.ascontiguousarray(
            w2e.reshape(NK, 128, DIM).transpose(1, 0, 2).reshape(128, NK * DIM)
        ).astype(BF)

        # packed small params: [dw0|dw1|b0|b1|g0] as (128, 16) column groups
        cols = []
        for nm in (f'dw{f}_0', f'dw{f}_1'):
            cols.append(np.asarray(inputs[nm])[:, 0].reshape(NK, 128).T)
        for nm in (f'b{f}_0', f'b{f}_1'):
            cols.append(np.asarray(inputs[nm]).reshape(NK, 128).T)
        cols.append(W0[np.arange(DH), blk].reshape(NK, 128).T)
        fl[f'sml{f}'] = np.ascontiguousarray(
            np.concatenate(cols, axis=1).astype(np.float32))
        fl[f'sml64{f}'] = np.ascontiguousarray(np.stack(
            [np.asarray(inputs[f'dw{f}_2'])[:, 0],
             np.asarray(inputs[f'b{f}_2'])], axis=1).astype(np.float32))

    fl['gatec0'] = np.full((DIM, 1), float(np.asarray(inputs['gate0'])[0]), np.float32)
    fl['gatec1'] = np.full((DIM, 1), float(np.asarray(inputs['gate1'])[0]), np.float32)
    fl['flip64'] = np.eye(DIM, dtype=np.float32)[:, ::-1].astype(BF)
    fl['ident128'] = np.eye(128, dtype=np.float32)
    fl['ones64'] = np.ones((DIM, 1), np.float32)
    fl['onescol'] = np.ones((128, 1), np.float32)
    return fl


# ------------------------------------------------- walrus sync-wait splitter
def _split_sync_waits(nc, max_waits=1):
    import concourse.mybir as mybir
    for func in nc.m.functions:
        for blkb in func.blocks:
            insts = list(blkb.instructions)
            out = []
            changed = False
            for inst in insts:
                si = inst.sync_info
                if si is not None and len(si.on_wait) > max_waits:
                    waits = list(si.on_wait)
                    keep, pre = waits[-max_waits:], waits[:-max_waits]
                    chunks = [pre[i:i + max_waits] for i in range(0, len(pre), max_waits)]
                    for ci, chunk in enumerate(chunks):
                        nop = mybir.InstNoOp(name=f"{inst.name}.w{ci}", ins=[], outs=[])
                        nop.engine = inst.engine
                        nop.sync_info = mybir.SyncInfo(on_wait=chunk, on_update=[])
                        out.append(nop)
                    inst.sync_info = mybir.SyncInfo(
                        on_wait=keep, on_update=list(si.on_update))
                    changed = True
                out.append(inst)
            if changed:
                try:
                    blkb.instructions = out
                except Exception:
                    while len(blkb.instructions):
                        blkb.remove_instruction(blkb.instructions[-1])
                    for i2 in out:
                        blkb.add_instruction(i2)


# ---------------------------------------------------------------- bass build
def _build(main_dt_name, repeat=1):
    import concourse.bass as bass
    import concourse.mybir as mybir
    import concourse.tile as tile

    f32 = mybir.dt.float32
    bf16 = mybir.dt.bfloat16
    AO = mybir.AluOpType
    AF = mybir.ActivationFunctionType

    nc = bass.Bass()
    P = {}

    def dram(name, shape, dt=f32):
        P[name] = nc.declare_dram_parameter(name, list(shape), dt, isOutput=False)

    dram('xT', (DIM, BC))
    for f in range(3):
        dram(f'w0raw{f}', (DIM, DH), bf16); dram(f'w0exp{f}', (DIM, DH), bf16)
        dram(f'w1cat{f}', (128, W1COLS), bf16)
        dram(f'w1dexp{f}', (128, DH), bf16)
        dram(f'w2raw{f}', (128, NK * DIM), bf16)
        dram(f'w2exp{f}', (128, NK * DIM), bf16)
        dram(f'sml{f}', (128, 5 * NK))
        dram(f'sml64{f}', (DIM, 2))
    dram('gatec0', (DIM, 1)); dram('gatec1', (DIM, 1))
    dram('flip64', (DIM, DIM), bf16); dram('ident128', (128, 128))
    dram('ones64', (DIM, 1)); dram('onescol', (128, 1))
    out = nc.declare_dram_parameter('out', [1, BC], f32, isOutput=True)

    with tile.TileContext(nc) as tc:
        with tc.tile_pool(name='const', bufs=1) as cpool, \
             tc.tile_pool(name='wchunk', bufs=2) as wpool, \
             tc.tile_pool(name='acts', bufs=1) as apool, \
             tc.tile_pool(name='sq', bufs=1) as sqpool, \
             tc.tile_pool(name='small', bufs=2) as spool, \
             tc.tile_pool(name='psz', bufs=2, space='PSUM') as pp, \
             tc.tile_pool(name='pse', bufs=2, space='PSUM') as pp1, \
             tc.tile_pool(name='psw', bufs=2, space='PSUM') as ppw:

            def mload(pool, tag, shape, dt, srcap, bufs=None):
                t = pool.tile(list(shape), dt, name=tag, tag=tag, bufs=bufs)
                if dt != srcap.tensor.dtype:
                    nc.gpsimd.dma_start(out=t[:], in_=srcap)
                else:
                    nc.sync.dma_start(out=t[:], in_=srcap)
                return t

            ones64 = mload(cpool, 'ones64', (DIM, 1), f32, P['ones64'][:])
            onescol_bf = mload(cpool, 'onescol_bf', (128, 1), bf16, P['onescol'][:])
            flip64 = mload(cpool, 'flip64', (DIM, DIM), bf16, P['flip64'][:])
            ident = mload(cpool, 'ident', (128, 128), f32, P['ident128'][:])
            xT = mload(cpool, 'xT', (DIM, BC), bf16, P['xT'][:])

            for _rep in range(repeat):
              if _rep:
                xT = mload(cpool, 'xT', (DIM, BC), bf16, P['xT'][:])
              acc = cpool.tile([DIM, BC], f32, name='acc', tag='acc')
              nc.vector.memset(acc[:], 0.0)
              E2s = [cpool.tile([DIM, BC], f32, name=f'E2_{f}', tag=f'E2_{f}') for f in range(3)]
              egs = [cpool.tile([DIM, 1], f32, name=f'eg_{f}', tag=f'eg_{f}') for f in range(2)]
              sgs = []

              # gate scalars for both flows up-front (exp+tanh co-resident)
              for f in range(2):
                    gc = mload(spool, f'gc{f}', (DIM, 1), f32, P[f'gatec{f}'][:])
                    th = spool.tile([DIM, 1], f32, name=f'th{f}', tag=f'th{f}')
                    nc.scalar.activation(th[:], gc[:], AF.Tanh, scale=0.5)
                    sg = spool.tile([DIM, 1], f32, name=f'sg{f}', tag=f'sg{f}')
                    nc.vector.tensor_scalar(out=sg[:], in0=th[:], scalar1=0.5,
                                            scalar2=0.5, op0=AO.mult, op1=AO.add)
                    omsg = spool.tile([DIM, 1], f32, name=f'omsg{f}', tag=f'omsg{f}')
                    nc.vector.tensor_scalar(out=omsg[:], in0=sg[:], scalar1=-1.0,
                                            scalar2=1.0, op0=AO.mult, op1=AO.add)
                    nc.scalar.activation(egs[f][:], gc[:], AF.Exp)
                    sgs.append((sg, omsg))

              for f in range(3):
                    # ---- loads (all via sync/HWDGE)
                    w0raw = mload(apool, 'w0raw', (DIM, DH), bf16, P[f'w0raw{f}'][:])
                    w0exp_in = mload(apool, 'w0expin', (DIM, DH), bf16, P[f'w0exp{f}'][:])
                    w1cat = mload(wpool, 'w1cat', (128, W1COLS), bf16, P[f'w1cat{f}'][:])
                    w1dexp_in = mload(apool, 'w1dexpin', (128, DH), bf16, P[f'w1dexp{f}'][:])
                    w2raw = mload(apool, 'w2raw', (128, NK * DIM), bf16, P[f'w2raw{f}'][:])
                    w2exp_in = mload(apool, 'w2expin', (128, NK * DIM), bf16, P[f'w2exp{f}'][:])
                    sml = mload(spool, 'sml', (128, 5 * NK), f32, P[f'sml{f}'][:])
                    sml64 = mload(spool, 'sml64', (DIM, 2), f32, P[f'sml64{f}'][:])
                    dw0, dw1 = sml[:, 0:NK], sml[:, NK:2 * NK]
                    b0, b1 = sml[:, 2 * NK:3 * NK], sml[:, 3 * NK:4 * NK]
                    g0c = sml[:, 4 * NK:5 * NK]
                    dw2, b2 = sml64[:, 0:1], sml64[:, 1:2]

                    # ---- device exps (exp table) + fused tiles
                    w0expt = apool.tile([DIM, DH], bf16, name='w0expt', tag='w0expt')
                    nc.scalar.activation(w0expt[:], w0exp_in[:], AF.Exp)
                    w0all = apool.tile([DIM, DH], bf16, name='w0all', tag='w0all')
                    nc.gpsimd.tensor_tensor(w0all[:], w0raw[:], w0expt[:], AO.add)
                    dexpe = apool.tile([128, DH], bf16, name='dexpe', tag='dexpe')
                    nc.scalar.activation(dexpe[:], w1dexp_in[:], AF.Exp)
                    # fold exp'd 32-diag blocks into w1cat's in-chunk columns
                    # (those slots hold only the strictly-lower 32-blocks; the
                    # 32-diag positions are structurally zero, so add == insert)
                    for k in range(NK):
                        nc.gpsimd.tensor_tensor(
                            w1cat[:, W1OFF[k]:W1OFF[k] + 128],
                            w1cat[:, W1OFF[k]:W1OFF[k] + 128],
                            dexpe[:, 128 * k:128 * k + 128], AO.add)
                    w2e = apool.tile([128, NK * DIM], bf16, name='w2e', tag='w2e')
                    nc.scalar.activation(w2e[:], w2exp_in[:], AF.Exp)
                    w2all = apool.tile([128, NK * DIM], bf16, name='w2all', tag='w2all')
                    nc.gpsimd.tensor_tensor(w2all[:], w2raw[:], w2e[:], AO.add)
                    eg0 = spool.tile([128, NK], f32, name='eg0', tag='eg0')
                    nc.scalar.activation(eg0[:], g0c, AF.Exp)

                    # ---- wsn via Gram matmuls + diagonal extraction
                    # (mask diag with identity on DVE; column sums of the
                    # masked Gram equal its diagonal, so one N=1 matmul against
                    # a ones column drops the diagonal into the wsn psum)
                    wsnp = ppw.tile([128, 33], f32, name='wsnp', tag='wsnp',
                                    bufs=1)

                    def diag_extract(G, n, wsncol):
                        gm = sqpool.tile([128, 128], bf16, name='gscr', tag='gscr',
                                         bufs=2)
                        nc.vector.tensor_tensor(gm[:n, :n], G[:], ident[:n, :n],
                                                AO.mult)
                        nc.tensor.matmul(wsncol, gm[:n, :n], onescol_bf[:n, :],
                                         start=True, stop=True)

                    for m in range(NK):
                        G = ppw.tile([128, 128], f32, name='g1', tag='g1')
                        sl = w0all[:, 128 * m:128 * m + 128]
                        nc.tensor.matmul(G[:], sl, sl, start=True, stop=True)
                        diag_extract(G, 128, wsnp[:, m:m + 1])

                    for m in range(NK):
                        G = ppw.tile([128, 128], f32, name='g1', tag='g1')
                        for k in range(m + 1):
                            sl = w1cat[:, W1OFF[k] + 128 * (m - k):
                                       W1OFF[k] + 128 * (m - k) + 128]
                            nc.tensor.matmul(G[:], sl, sl, start=(k == 0),
                                             stop=(k == m))
                        diag_extract(G, 128, wsnp[:, NK + m:NK + m + 1])

                    # ---- wsn2 via Gram
                    G2 = ppw.tile([DIM, DIM], f32, name='g2', tag='g1')
                    for k in range(NK):
                        sl = w2all[:, k * DIM:(k + 1) * DIM]
                        nc.tensor.matmul(G2[:], sl, sl, start=(k == 0), stop=(k == NK - 1))
                    diag_extract(G2, DIM, wsnp[:DIM, 32:33])

                    # ---- s-phase (ln+exp grouped: 2 table swaps per flow)
                    s0 = _scol(nc, spool, AO, AF, wsnp[:, 0:NK], dw0, (128, NK), 's0')
                    s1 = _scol(nc, spool, AO, AF, wsnp[:, NK:2 * NK], dw1, (128, NK), 's1')
                    s2 = _scol(nc, spool, AO, AF, wsnp[:DIM, 32:33], dw2, (DIM, 1), 's2')

                    c0 = spool.tile([128, NK], f32, name='c0', tag='c0')
                    nc.vector.tensor_tensor(c0[:], eg0[:], s0[:], AO.mult)
                    c0n = spool.tile([128, NK], f32, name='c0n', tag='c0n')
                    nc.vector.tensor_scalar(out=c0n[:], in0=c0[:], scalar1=-1.0,
                                            scalar2=None, op0=AO.mult)
                    s1n = spool.tile([128, NK], f32, name='s1n', tag='s1n')
                    nc.vector.tensor_scalar(out=s1n[:], in0=s1[:], scalar1=-1.0,
                                            scalar2=None, op0=AO.mult)

                    # ---- layer 0: matmul -> DVE affine -> coarse tanh + square
                    zs0 = apool.tile([128, DH], f32, name='zs0', tag='zs0')
                    h0 = apool.tile([128, DH], bf16, name='h0', tag='h0')
                    hq0 = apool.tile([128, DH], bf16, name='hq0', tag='hq0')
                    E0 = apool.tile([128, DH], bf16, name='E0', tag='E0')
                    for g in range(4):
                        ps = pp.tile([128, 512], f32, name='zmain', tag='zmain')
                        for j in range(4):
                            m = 4 * g + j
                            nc.tensor.matmul(ps[:, 128 * j:128 * j + 128],
                                             w0all[:, 128 * m:128 * m + 128], xT[:],
                                             start=True, stop=True)
                        for j in range(4):
                            m = 4 * g + j
                            nc.vector.tensor_scalar(
                                out=zs0[:, 128 * m:128 * m + 128],
                                in0=ps[:, 128 * j:128 * j + 128],
                                scalar1=s0[:, m:m + 1], scalar2=b0[:, m:m + 1],
                                op0=AO.mult, op1=AO.add)
                        gsl = slice(512 * g, 512 * g + 512)
                        nc.scalar.activation(h0[:, gsl], zs0[:, gsl], AF.Tanh)
                        nc.scalar.square(hq0[:, gsl], h0[:, gsl])
                        for j in range(4):
                            m = 4 * g + j
                            # E0 = c0 * (1 - h0^2)
                            nc.vector.tensor_scalar(
                                out=E0[:, 128 * m:128 * m + 128],
                                in0=hq0[:, 128 * m:128 * m + 128],
                                scalar1=c0n[:, m:m + 1], scalar2=c0[:, m:m + 1],
                                op0=AO.mult, op1=AO.add)

                    # ---- layer 1 (+ E-path combine)
                    zs1 = apool.tile([128, DH], f32, name='zs1', tag='zs1')
                    h1 = apool.tile([128, DH], bf16, name='h1', tag='h1')
                    hq1 = apool.tile([128, DH], bf16, name='hq1', tag='hq1')
                    ets = apool.tile([128, DH], bf16, name='ets', tag='ets')
                    E1 = apool.tile([128, DH], bf16, name='E1', tag='E1')
                    for g in range(4):
                        ps = pp.tile([128, 512], f32, name='zmain', tag='zmain')
                        for j in range(4):
                            m = 4 * g + j
                            sl = ps[:, 128 * j:128 * j + 128]
                            for k in range(m + 1):
                                nc.tensor.matmul(
                                    sl, w1cat[:, W1OFF[k] + 128 * (m - k):
                                              W1OFF[k] + 128 * (m - k) + 128],
                                    h0[:, 128 * k:128 * k + 128],
                                    start=(k == 0), stop=(k == m))
                        psE = pp1.tile([128, 512], f32, name='epath', tag='epath')
                        for j in range(4):
                            m = 4 * g + j
                            nc.tensor.matmul(psE[:, 128 * j:128 * j + 128],
                                             dexpe[:, 128 * m:128 * m + 128],
                                             E0[:, 128 * m:128 * m + 128],
                                             start=True, stop=True)
                        for j in range(4):
                            m = 4 * g + j
                            nc.vector.tensor_scalar(
                                out=zs1[:, 128 * m:128 * m + 128],
                                in0=ps[:, 128 * j:128 * j + 128],
                                scalar1=s1[:, m:m + 1], scalar2=b1[:, m:m + 1],
                                op0=AO.mult, op1=AO.add)
                        gsl = slice(512 * g, 512 * g + 512)
                        nc.scalar.activation(h1[:, gsl], zs1[:, gsl], AF.Tanh)
                        nc.scalar.square(hq1[:, gsl], h1[:, gsl])
                        for j in range(4):
                            m = 4 * g + j
                            # ets = s1 * (1 - h1^2)
                            nc.vector.tensor_scalar(
                                out=ets[:, 128 * m:128 * m + 128],
                                in0=hq1[:, 128 * m:128 * m + 128],
                                scalar1=s1n[:, m:m + 1], scalar2=s1[:, m:m + 1],
                                op0=AO.mult, op1=AO.add)
                        nc.vector.tensor_tensor(E1[:, gsl], psE[:], ets[:, gsl],
                                                AO.mult)

                    # ---- layer 2
                    psz2 = pp.tile([DIM, BC], f32, name='zsmall', tag='zsmall',
                                   bufs=1)
                    for k in range(NK):
                        nc.tensor.matmul(psz2[:], w2all[:, k * DIM:(k + 1) * DIM],
                                         h1[:, 128 * k:128 * k + 128],
                                         start=(k == 0), stop=(k == NK - 1))
                    z2 = spool.tile([DIM, BC], f32, name='z2s', tag='z2s')
                    nc.scalar.activation(z2[:], psz2[:], AF.Identity,
                                         bias=b2, scale=s2[:, 0:1])
                    psE2 = pp1.tile([DIM, BC], f32, name='e2small', tag='epath')
                    for k in range(NK):
                        nc.tensor.matmul(psE2[:], w2e[:, k * DIM:(k + 1) * DIM],
                                         E1[:, 128 * k:128 * k + 128],
                                         start=(k == 0), stop=(k == NK - 1))
                    nc.vector.tensor_scalar(out=E2s[f][:], in0=psE2[:],
                                            scalar1=s2[:, 0:1], scalar2=None,
                                            op0=AO.mult)

                    # ---- gate mix / flip or final logp term
                    if f < 2:
                        sg, omsg = sgs[f]
                        ta = spool.tile([DIM, BC], f32, name='ta', tag='ta')
                        nc.gpsimd.tensor_scalar(out=ta[:], in0=z2[:], scalar1=sg[:],
                                                scalar2=None, op0=AO.mult)
                        tb = spool.tile([DIM, BC], f32, name='tb', tag='tb')
                        nc.gpsimd.tensor_scalar(out=tb[:], in0=xT[:], scalar1=omsg[:],
                                                scalar2=None, op0=AO.mult)
                        xmix = spool.tile([DIM, BC], bf16, name='xmix', tag='xmix')
                        nc.gpsimd.tensor_tensor(xmix[:], ta[:], tb[:], AO.add)
                        psf = pp.tile([DIM, BC], f32, name='zsmall', tag='zsmall',
                                      bufs=1)
                        nc.tensor.matmul(psf[:], flip64[:], xmix[:], start=True, stop=True)
                        nc.scalar.activation(xT[:], psf[:], AF.Copy)
                    else:
                        sqx = spool.tile([DIM, BC], f32, name='sqx', tag='sqx')
                        nc.scalar.square(sqx[:], z2[:])
                        nc.gpsimd.tensor_scalar(out=sqx[:], in0=sqx[:], scalar1=-0.5,
                                                scalar2=-0.5 * LOG_2PI, op0=AO.mult, op1=AO.add)
                        nc.gpsimd.tensor_tensor(acc[:], acc[:], sqx[:], AO.add)

              # ---- ldj tail (ln phase)
              for f in range(2):
                    u = spool.tile([DIM, BC], f32, name='u', tag='u')
                    nc.gpsimd.tensor_scalar(out=u[:], in0=E2s[f][:], scalar1=egs[f][:],
                                            scalar2=1.0, op0=AO.mult, op1=AO.add)
                    lf = spool.tile([DIM, BC], f32, name='lf', tag='lf')
                    nc.scalar.activation(lf[:], u[:], AF.Ln)
                    l1p = spool.tile([DIM, 1], f32, name='l1p', tag='l1p')
                    nc.vector.tensor_scalar(out=l1p[:], in0=egs[f][:], scalar1=1.0,
                                            scalar2=None, op0=AO.add)
                    nc.scalar.activation(l1p[:], l1p[:], AF.Ln)
                    nc.gpsimd.tensor_scalar(out=lf[:], in0=lf[:], scalar1=l1p[:],
                                            scalar2=None, op0=AO.subtract)
                    nc.gpsimd.tensor_tensor(acc[:], acc[:], lf[:], AO.add)
              lf2 = spool.tile([DIM, BC], f32, name='lf2', tag='lf2')
              nc.scalar.activation(lf2[:], E2s[2][:], AF.Ln)
              nc.gpsimd.tensor_tensor(acc[:], acc[:], lf2[:], AO.add)

              psum_out = pp.tile([1, BC], f32, name='zsmall', tag='zsmall',
                                 bufs=1)
              nc.tensor.matmul(psum_out[:], ones64[:], acc[:], start=True, stop=True)
              outs = spool.tile([1, BC], f32, name='outs', tag='outs')
              nc.vector.tensor_copy(outs[:], psum_out[:])
              nc.sync.dma_start(out=out[:], in_=outs[:])

    _split_sync_waits(nc)
    return nc


def _scol(nc, spool, AO, AF, wsn, dwc, shape, tag):
    """s = exp(dw - 0.5*ln(wsn)) as (P, ncols) tile."""
    import concourse.mybir as mybir
    f32 = mybir.dt.float32
    ln = spool.tile(list(shape), f32, name=tag + 'n' + 'ln')
    nc.scalar.activation(ln[:], wsn[:], AF.Ln)
    t = spool.tile(list(shape), f32, name=tag + 'n' + 't')
    nc.vector.tensor_scalar(out=t[:], in0=ln[:], scalar1=-0.5, scalar2=None, op0=AO.mult)
    nc.vector.tensor_tensor(t[:], t[:], dwc, AO.add)
    s = spool.tile(list(shape), f32, name=tag + 'n' + 's')
    nc.scalar.activation(s[:], t[:], AF.Exp)
    return s


# ------------------------------------------------------------------ runner
def _make_runner(nc, n_cores):
    import jax
    from jax.sharding import Mesh, PartitionSpec
    from jax.experimental.shard_map import shard_map
    import concourse.mybir as mybir
    from concourse.bass2jax import (_bass_exec_p, partition_id_tensor,
                                    install_neuronx_cc_hook)
    install_neuronx_cc_hook()
    partition_name = nc.partition_id_tensor.name if nc.partition_id_tensor else None
    in_names, out_names, out_avals = [], [], []
    for alloc in nc.m.functions[0].allocations:
        if not isinstance(alloc, mybir.MemoryLocationSet):
            continue
        name = alloc.memorylocations[0].name
        if alloc.kind == "ExternalInput":
            if name != partition_name:
                in_names.append(name)
        elif alloc.kind == "ExternalOutput":
            out_names.append(name)
            out_avals.append(jax.core.ShapedArray(
                tuple(alloc.tensor_shape), mybir.dt.np(alloc.dtype)))
    n_params = len(in_names)
    all_names = in_names + out_names + ([partition_name] if partition_name else [])

    def _body(*args):
        operands = list(args)
        if partition_name is not None:
            operands.append(partition_id_tensor())
        outs = _bass_exec_p.bind(
            *operands, out_avals=tuple(out_avals), in_names=tuple(all_names),
            out_names=tuple(out_names), lowering_input_output_aliases=(),
            sim_require_finite=False, sim_require_nnan=False, nc=nc)
        return tuple(outs)

    devices = jax.devices()[:n_cores]
    mesh = Mesh(np.asarray(devices), ("core",))
    n_outs = len(out_names)
    in_specs = (PartitionSpec("core"),) * (n_params + n_outs)
    out_specs = (PartitionSpec("core"),) * n_outs
    fn = jax.jit(shard_map(_body, mesh=mesh, in_specs=in_specs,
                           out_specs=out_specs, check_rep=False),
                 keep_unused=True)
    return fn, in_names, out_names, out_avals


def _get_runner():
    key = ('runner', MAIN_DT)
    if key not in _CACHE:
        import sys, os
        d = os.path.dirname(os.path.abspath(__file__))
        if d not in sys.path:
            sys.path.insert(0, d)
        nc = _build(MAIN_DT)
        _CACHE[key] = _make_runner(nc, NCORES)
    return _CACHE[key]


def kernel(**inputs):
    fl = _host_prep(inputs)
    x = np.asarray(inputs['x'])
    fn, in_names, out_names, out_avals = _get_runner()
    in_maps = []
    for c in range(NCORES):
        m = dict(fl)
        m['xT'] = np.ascontiguousarray(x[c * BC:(c + 1) * BC, :].T)
        in_maps.append(m)
    concat_in = [np.concatenate([np.asarray(m[name]) for m in in_maps], axis=0)
                 for name in in_names]
    concat_zeros = [np.zeros((NCORES * a.shape[0], *a.shape[1:]), a.dtype)
                    for a in out_avals]
    outs = fn(*concat_in, *concat_zeros)
    o = np.asarray(outs[0]).reshape(NCORES, BC)
    return o.reshape(B).astype(np.float32)


# revision 44
# speedup vs baseline: 1.8215x; 1.2256x over previous
"""BNAF forward + log-det on 8 TRN2 NeuronCores (self-contained).

Sharding: data-parallel over batch (128 rows/core), params replicated.
Host does layout-only prep (transpose / gather / structural masking / constant
fills / dtype storage casts); all arithmetic on input values happens on device.

Math restructuring (validated vs reference in numpy, rel err ~1e-3 bf16):
  masked weight: w = raw-lower-blocks + exp(diag-blocks);  out = (x @ w.T)*s + b
  with s[r] = exp(dw[r] - 0.5*ln(wsn[r])), wsn = row sums of w^2.
  Jacobian log-det chain in linear domain:
    exp(log|dtanh|) = 1 - tanh^2
    E0 = wn0_diag*(1-h0^2); E1 = (blockdiag(wn1_exp)@E0)*s1*(1-h1^2)
    E2 = s2*blockdiag(wn2_exp)@E1
    ldj = sum_d [ln(1 + e^gate * E2) - ln(1 + e^gate)] (flows 0,1); sum ln E2 (flow 2)

Perf structure (measured ~80-95us/body vs 330us baseline):
  - all matmuls bf16: f32r at free-dim 128 runs quarter-rate on the PE, and
    bf16 DRAM storage halves HBM traffic (~18MB/body/core)
  - W1's strictly-lower 32-blocks and exp'd 32-diag blocks share one
    concatenated triangular tensor (one DMA); the exp'd diag is added into
    its structurally-zero slots on device
  - wsn row-norms via Gram matmuls (slice.T @ slice accumulated in PSUM);
    the diagonal is extracted by masking with an identity on DVE and
    column-summing the masked Gram with one N=1 matmul (column sums of a
    diag-masked matrix ARE its diagonal); W1 diag exp stored compact (32 rows)
  - z scale/bias on DVE from PSUM; tanh + h^2 as coarse 512-col activations
  - activation-table discipline: ln/exp grouped per flow (table load costs
    1.28us; ln never co-resides with tanh)
  - in-order engine queues make op placement critical: the inter-flow tail
    (z2 -> xmix -> flip -> xT) stays on act/Pool/PE clear of next-flow DVE
    work (a lookahead-prep variant regressed 77->129us from exactly this)
"""
import numpy as np
import ml_dtypes

BF = ml_dtypes.bfloat16
DIM, HID, B = 64, 32, 1024
NCORES = 8
BC = B // NCORES
DH = DIM * HID             # 2048
NK = DH // 128             # 16
W1COLS = sum(DH - 128 * k for k in range(NK))   # 17408
W1OFF = [2048 * k - 64 * k * (k - 1) for k in range(NK)]
LOG_2PI = float(np.log(2.0 * np.pi))
NEG = -1e30
MAIN_DT = 'bfloat16'

_CACHE = {}


# ---------------------------------------------------------------- host prep
def _host_prep(inputs):
    fl = {}
    r = np.arange(DH)
    blk = r // HID
    c64 = np.arange(DIM)
    i128 = np.arange(128)
    for f in range(3):
        W0 = np.asarray(inputs[f'W{f}_0'])
        W1 = np.asarray(inputs[f'W{f}_1'])
        W2 = np.asarray(inputs[f'W{f}_2'])

        W0T = np.ascontiguousarray(W0.T)                      # (64, 2048)
        keep = c64[:, None] < blk[None, :]
        diag = c64[:, None] == blk[None, :]
        fl[f'w0raw{f}'] = np.where(keep, W0T, 0.0).astype(BF)
        fl[f'w0exp{f}'] = np.where(diag, W0T, NEG).astype(BF)

        W1T = np.ascontiguousarray(W1.T)                      # (2048, 2048)
        keep1 = blk[:, None] < blk[None, :]                   # all lower 32-blocks
        w1m = np.where(keep1, W1T, 0.0).astype(BF)
        fl[f'w1cat{f}'] = np.concatenate(
            [w1m[128 * k:128 * k + 128, 128 * k:] for k in range(NK)], axis=1)
        d_exp = np.full((128, DH), NEG, np.float32)
        dia = (i128[:, None] // HID) == (i128[None, :] // HID)
        for k in range(NK):
            t = W1T[128 * k:128 * k + 128, 128 * k:128 * k + 128]
            d_exp[:, 128 * k:128 * k + 128] = np.where(dia, t, NEG)
        fl[f'w1dexp{f}'] = d_exp.astype(BF)

        W2T = np.ascontiguousarray(W2.T)                      # (2048, 64)
        keep2 = c64[None, :] > blk[:, None]
        diag2 = c64[None, :] == blk[:, None]
        w2r = np.where(keep2, W2T, 0.0).astype(np.float32)
        fl[f'w2raw{f}'] = np.ascontiguousarray(
            w2r.reshape(NK, 128, DIM).transpose(1, 0, 2).reshape(128, NK * DIM)
        ).astype(BF)
        w2e = np.where(diag2, W2T, NEG).astype(np.float32)
        fl[f'w2exp{f}'] = np.ascontiguousarray(
            w2e.reshape(NK, 128, DIM).transpose(1, 0, 2).reshape(128, NK * DIM)
        ).astype(BF)

        # packed small params: [dw0|dw1|b0|b1|g0] as (128, 16) column groups
        cols = []
        for nm in (f'dw{f}_0', f'dw{f}_1'):
            cols.append(np.asarray(inputs[nm])[:, 0].reshape(NK, 128).T)
        for nm in (f'b{f}_0', f'b{f}_1'):
            cols.append(np.asarray(inputs[nm]).reshape(NK, 128).T)
        cols.append(W0[np.arange(DH), blk].reshape(NK, 128).T)
        fl[f'sml{f}'] = np.ascontiguousarray(
            np.concatenate(cols, axis=1).astype(np.float32))
        fl[f'sml64{f}'] = np.ascontiguousarray(np.stack(
            [np.asarray(inputs[f'dw{f}_2'])[:, 0],
             np.asarray(inputs[f'b{f}_2'])], axis=1).astype(np.float32))

    fl['gatec0'] = np.full((DIM, 1), float(np.asarray(inputs['gate0'])[0]), np.float32)
    fl['gatec1'] = np.full((DIM, 1), float(np.asarray(inputs['gate1'])[0]), np.float32)
    fl['flip64'] = np.eye(DIM, dtype=np.float32)[:, ::-1].astype(BF)
    fl['ident128'] = np.eye(128, dtype=np.float32)
    fl['ones64'] = np.ones((DIM, 1), np.float32)
    fl['onescol'] = np.ones((128, 1), np.float32)
    return fl


# ------------------------------------------------- walrus sync-wait splitter
def _split_sync_waits(nc, max_waits=1):
    import concourse.mybir as mybir
    for func in nc.m.functions:
        for blkb in func.blocks:
            insts = list(blkb.instructions)
            out = []
            changed = False
            for inst in insts:
                si = inst.sync_info
                if si is not None and len(si.on_wait) > max_waits:
                    waits = list(si.on_wait)
                    keep, pre = waits[-max_waits:], waits[:-max_waits]
                    chunks = [pre[i:i + max_waits] for i in range(0, len(pre), max_waits)]
                    for ci, chunk in enumerate(chunks):
                        nop = mybir.InstNoOp(name=f"{inst.name}.w{ci}", ins=[], outs=[])
                        nop.engine = inst.engine
                        nop.sync_info = mybir.SyncInfo(on_wait=chunk, on_update=[])
                        out.append(nop)
                    inst.sync_info = mybir.SyncInfo(
                        on_wait=keep, on_update=list(si.on_update))
                    changed = True
                out.append(inst)
            if changed:
                try:
                    blkb.instructions = out
                except Exception:
                    while len(blkb.instructions):
                        blkb.remove_instruction(blkb.instructions[-1])
                    for i2 in out:
                        blkb.add_instruction(i2)


# ---------------------------------------------------------------- bass build
def _build(main_dt_name, repeat=1):
    import concourse.bass as bass
    import concourse.mybir as mybir
    import concourse.tile as tile

    f32 = mybir.dt.float32
    bf16 = mybir.dt.bfloat16
    AO = mybir.AluOpType
    AF = mybir.ActivationFunctionType

    nc = bass.Bass()
    P = {}

    def dram(name, shape, dt=f32):
        P[name] = nc.declare_dram_parameter(name, list(shape), dt, isOutput=False)

    dram('xT', (DIM, BC))
    for f in range(3):
        dram(f'w0raw{f}', (DIM, DH), bf16); dram(f'w0exp{f}', (DIM, DH), bf16)
        dram(f'w1cat{f}', (128, W1COLS), bf16)
        dram(f'w1dexp{f}', (128, DH), bf16)
        dram(f'w2raw{f}', (128, NK * DIM), bf16)
        dram(f'w2exp{f}', (128, NK * DIM), bf16)
        dram(f'sml{f}', (128, 5 * NK))
        dram(f'sml64{f}', (DIM, 2))
    dram('gatec0', (DIM, 1)); dram('gatec1', (DIM, 1))
    dram('flip64', (DIM, DIM), bf16); dram('ident128', (128, 128))
    dram('ones64', (DIM, 1)); dram('onescol', (128, 1))
    out = nc.declare_dram_parameter('out', [1, BC], f32, isOutput=True)

    with tile.TileContext(nc) as tc:
        with tc.tile_pool(name='const', bufs=1) as cpool, \
             tc.tile_pool(name='wchunk', bufs=2) as wpool, \
             tc.tile_pool(name='acts', bufs=1) as apool, \
             tc.tile_pool(name='sq', bufs=1) as sqpool, \
             tc.tile_pool(name='small', bufs=2) as spool, \
             tc.tile_pool(name='psz', bufs=2, space='PSUM') as pp, \
             tc.tile_pool(name='pse', bufs=2, space='PSUM') as pp1, \
             tc.tile_pool(name='psw', bufs=2, space='PSUM') as ppw:

            def mload(pool, tag, shape, dt, srcap, bufs=None):
                t = pool.tile(list(shape), dt, name=tag, tag=tag, bufs=bufs)
                if dt != srcap.tensor.dtype:
                    nc.gpsimd.dma_start(out=t[:], in_=srcap)
                else:
                    nc.sync.dma_start(out=t[:], in_=srcap)
                return t

            ones64 = mload(cpool, 'ones64', (DIM, 1), f32, P['ones64'][:])
            onescol_bf = mload(cpool, 'onescol_bf', (128, 1), bf16, P['onescol'][:])
            flip64 = mload(cpool, 'flip64', (DIM, DIM), bf16, P['flip64'][:])
            ident = mload(cpool, 'ident', (128, 128), f32, P['ident128'][:])
            xT = mload(cpool, 'xT', (DIM, BC), bf16, P['xT'][:])

            for _rep in range(repeat):
              if _rep:
                xT = mload(cpool, 'xT', (DIM, BC), bf16, P['xT'][:])
              acc = cpool.tile([DIM, BC], f32, name='acc', tag='acc')
              nc.vector.memset(acc[:], 0.0)
              E2s = [cpool.tile([DIM, BC], f32, name=f'E2_{f}', tag=f'E2_{f}') for f in range(3)]
              egs = [cpool.tile([DIM, 1], f32, name=f'eg_{f}', tag=f'eg_{f}') for f in range(2)]
              sgs = []

              # gate scalars for both flows up-front (exp+tanh co-resident)
              for f in range(2):
                    gc = mload(spool, f'gc{f}', (DIM, 1), f32, P[f'gatec{f}'][:])
                    th = spool.tile([DIM, 1], f32, name=f'th{f}', tag=f'th{f}')
                    nc.scalar.activation(th[:], gc[:], AF.Tanh, scale=0.5)
                    sg = spool.tile([DIM, 1], f32, name=f'sg{f}', tag=f'sg{f}')
                    nc.vector.tensor_scalar(out=sg[:], in0=th[:], scalar1=0.5,
                                            scalar2=0.5, op0=AO.mult, op1=AO.add)
                    omsg = spool.tile([DIM, 1], f32, name=f'omsg{f}', tag=f'omsg{f}')
                    nc.vector.tensor_scalar(out=omsg[:], in0=sg[:], scalar1=-1.0,
                                            scalar2=1.0, op0=AO.mult, op1=AO.add)
                    nc.scalar.activation(egs[f][:], gc[:], AF.Exp)
                    sgs.append((sg, omsg))

              for f in range(3):
                    # ---- loads (all via sync/HWDGE)
                    w0raw = mload(apool, 'w0raw', (DIM, DH), bf16, P[f'w0raw{f}'][:])
                    w0exp_in = mload(apool, 'w0expin', (DIM, DH), bf16, P[f'w0exp{f}'][:])
                    w1cat = mload(wpool, 'w1cat', (128, W1COLS), bf16, P[f'w1cat{f}'][:])
                    w1dexp_in = mload(apool, 'w1dexpin', (128, DH), bf16, P[f'w1dexp{f}'][:])
                    w2raw = mload(apool, 'w2raw', (128, NK * DIM), bf16, P[f'w2raw{f}'][:])
                    w2exp_in = mload(apool, 'w2expin', (128, NK * DIM), bf16, P[f'w2exp{f}'][:])
                    sml = mload(spool, 'sml', (128, 5 * NK), f32, P[f'sml{f}'][:])
                    sml64 = mload(spool, 'sml64', (DIM, 2), f32, P[f'sml64{f}'][:])
                    dw0, dw1 = sml[:, 0:NK], sml[:, NK:2 * NK]
                    b0, b1 = sml[:, 2 * NK:3 * NK], sml[:, 3 * NK:4 * NK]
                    g0c = sml[:, 4 * NK:5 * NK]
                    dw2, b2 = sml64[:, 0:1], sml64[:, 1:2]

                    # ---- device exps (exp table) + fused tiles
                    w0expt = apool.tile([DIM, DH], bf16, name='w0expt', tag='w0expt')
                    nc.scalar.activation(w0expt[:], w0exp_in[:], AF.Exp)
                    w0all = apool.tile([DIM, DH], bf16, name='w0all', tag='w0all')
                    nc.gpsimd.tensor_tensor(w0all[:], w0raw[:], w0expt[:], AO.add)
                    dexpe = apool.tile([128, DH], bf16, name='dexpe', tag='dexpe')
                    nc.scalar.activation(dexpe[:], w1dexp_in[:], AF.Exp)
                    # fold exp'd 32-diag blocks into w1cat's in-chunk columns
                    # (those slots hold only the strictly-lower 32-blocks; the
                    # 32-diag positions are structurally zero, so add == insert)
                    for k in range(NK):
                        nc.gpsimd.tensor_tensor(
                            w1cat[:, W1OFF[k]:W1OFF[k] + 128],
                            w1cat[:, W1OFF[k]:W1OFF[k] + 128],
                            dexpe[:, 128 * k:128 * k + 128], AO.add)
                    w2e = apool.tile([128, NK * DIM], bf16, name='w2e', tag='w2e')
                    nc.scalar.activation(w2e[:], w2exp_in[:], AF.Exp)
                    w2all = apool.tile([128, NK * DIM], bf16, name='w2all', tag='w2all')
                    nc.gpsimd.tensor_tensor(w2all[:], w2raw[:], w2e[:], AO.add)
                    eg0 = spool.tile([128, NK], f32, name='eg0', tag='eg0')
                    nc.scalar.activation(eg0[:], g0c, AF.Exp)

                    # ---- wsn via Gram matmuls + diagonal extraction
                    # (mask diag with identity on DVE; column sums of the
                    # masked Gram equal its diagonal, so one N=1 matmul against
                    # a ones column drops the diagonal into the wsn psum)
                    wsnp = ppw.tile([128, 33], f32, name='wsnp', tag='wsnp',
                                    bufs=1)

                    def diag_extract(G, n, wsncol):
                        gm = sqpool.tile([128, 128], bf16, name='gscr', tag='gscr',
                                         bufs=2)
                        nc.vector.tensor_tensor(gm[:n, :n], G[:], ident[:n, :n],
                                                AO.mult)
                        nc.tensor.matmul(wsncol, gm[:n, :n], onescol_bf[:n, :],
                                         start=True, stop=True)

                    for m in range(NK):
                        G = ppw.tile([128, 128], f32, name='g1', tag='g1')
                        sl = w0all[:, 128 * m:128 * m + 128]
                        nc.tensor.matmul(G[:], sl, sl, start=True, stop=True)
                        diag_extract(G, 128, wsnp[:, m:m + 1])

                    for m in range(NK):
                        G = ppw.tile([128, 128], f32, name='g1', tag='g1')
                        for k in range(m + 1):
                            sl = w1cat[:, W1OFF[k] + 128 * (m - k):
                                       W1OFF[k] + 128 * (m - k) + 128]
                            nc.tensor.matmul(G[:], sl, sl, start=(k == 0),
                                             stop=(k == m))
                        diag_extract(G, 128, wsnp[:, NK + m:NK + m + 1])

                    # ---- wsn2 via Gram
                    G2 = ppw.tile([DIM, DIM], f32, name='g2', tag='g1')
                    for k in range(NK):
                        sl = w2all[:, k * DIM:(k + 1) * DIM]
                        nc.tensor.matmul(G2[:], sl, sl, start=(k == 0), stop=(k == NK - 1))
                    diag_extract(G2, DIM, wsnp[:DIM, 32:33])

                    # ---- s-phase (ln+exp grouped: 2 table swaps per flow)
                    s0 = _scol(nc, spool, AO, AF, wsnp[:, 0:NK], dw0, (128, NK), 's0')
                    s1 = _scol(nc, spool, AO, AF, wsnp[:, NK:2 * NK], dw1, (128, NK), 's1')
                    s2 = _scol(nc, spool, AO, AF, wsnp[:DIM, 32:33], dw2, (DIM, 1), 's2')

                    c0 = spool.tile([128, NK], f32, name='c0', tag='c0')
                    nc.vector.tensor_tensor(c0[:], eg0[:], s0[:], AO.mult)
                    c0n = spool.tile([128, NK], f32, name='c0n', tag='c0n')
                    nc.vector.tensor_scalar(out=c0n[:], in0=c0[:], scalar1=-1.0,
                                            scalar2=None, op0=AO.mult)
                    s1n = spool.tile([128, NK], f32, name='s1n', tag='s1n')
                    nc.vector.tensor_scalar(out=s1n[:], in0=s1[:], scalar1=-1.0,
                                            scalar2=None, op0=AO.mult)

                    # ---- layer 0: matmul -> DVE affine -> coarse tanh + square
                    zs0 = apool.tile([128, DH], f32, name='zs0', tag='zs0')
                    h0 = apool.tile([128, DH], bf16, name='h0', tag='h0')
                    hq0 = apool.tile([128, DH], bf16, name='hq0', tag='hq0')
                    E0 = apool.tile([128, DH], bf16, name='E0', tag='E0')
                    for g in range(4):
                        ps = pp.tile([128, 512], f32, name='zmain', tag='zmain')
                        for j in range(4):
                            m = 4 * g + j
                            nc.tensor.matmul(ps[:, 128 * j:128 * j + 128],
                                             w0all[:, 128 * m:128 * m + 128], xT[:],
                                             start=True, stop=True)
                        for j in range(4):
                            m = 4 * g + j
                            nc.vector.tensor_scalar(
                                out=zs0[:, 128 * m:128 * m + 128],
                                in0=ps[:, 128 * j:128 * j + 128],
                                scalar1=s0[:, m:m + 1], scalar2=b0[:, m:m + 1],
                                op0=AO.mult, op1=AO.add)
                        gsl = slice(512 * g, 512 * g + 512)
                        nc.scalar.activation(h0[:, gsl], zs0[:, gsl], AF.Tanh)
                        nc.scalar.square(hq0[:, gsl], h0[:, gsl])
                        for j in range(4):
                            m = 4 * g + j
                            # E0 = c0 * (1 - h0^2)
                            nc.vector.tensor_scalar(
                                out=E0[:, 128 * m:128 * m + 128],
                                in0=hq0[:, 128 * m:128 * m + 128],
                                scalar1=c0n[:, m:m + 1], scalar2=c0[:, m:m + 1],
                                op0=AO.mult, op1=AO.add)

                    # ---- layer 1 (+ E-path combine)
                    zs1 = apool.tile([128, DH], f32, name='zs1', tag='zs1')
                    h1 = apool.tile([128, DH], bf16, name='h1', tag='h1')
                    hq1 = apool.tile([128, DH], bf16, name='hq1', tag='hq1')
                    ets = apool.tile([128, DH], bf16, name='ets', tag='ets')
                    E1 = apool.tile([128, DH], bf16, name='E1', tag='E1')
                    for g in range(4):
                        ps = pp.tile([128, 512], f32, name='zmain', tag='zmain')
                        for j in range(4):
                            m = 4 * g + j
                            sl = ps[:, 128 * j:128 * j + 128]
                            for k in range(m + 1):
                                nc.tensor.matmul(
                                    sl, w1cat[:, W1OFF[k] + 128 * (m - k):
                                              W1OFF[k] + 128 * (m - k) + 128],
                                    h0[:, 128 * k:128 * k + 128],
                                    start=(k == 0), stop=(k == m))
                        psE = pp1.tile([128, 512], f32, name='epath', tag='epath')
                        for j in range(4):
                            m = 4 * g + j
                            nc.tensor.matmul(psE[:, 128 * j:128 * j + 128],
                                             dexpe[:, 128 * m:128 * m + 128],
                                             E0[:, 128 * m:128 * m + 128],
                                             start=True, stop=True)
                        for j in range(4):
                            m = 4 * g + j
                            nc.vector.tensor_scalar(
                                out=zs1[:, 128 * m:128 * m + 128],
                                in0=ps[:, 128 * j:128 * j + 128],
                                scalar1=s1[:, m:m + 1], scalar2=b1[:, m:m + 1],
                                op0=AO.mult, op1=AO.add)
                        gsl = slice(512 * g, 512 * g + 512)
                        nc.scalar.activation(h1[:, gsl], zs1[:, gsl], AF.Tanh)
                        nc.scalar.square(hq1[:, gsl], h1[:, gsl])
                        for j in range(4):
                            m = 4 * g + j
                            # ets = s1 * (1 - h1^2)
                            nc.vector.tensor_scalar(
                                out=ets[:, 128 * m:128 * m + 128],
                                in0=hq1[:, 128 * m:128 * m + 128],
                                scalar1=s1n[:, m:m + 1], scalar2=s1[:, m:m + 1],
                                op0=AO.mult, op1=AO.add)
                        nc.vector.tensor_tensor(E1[:, gsl], psE[:], ets[:, gsl],
                                                AO.mult)

                    # ---- layer 2
                    psz2 = pp.tile([DIM, BC], f32, name='zsmall', tag='zsmall',
                                   bufs=1)
                    for k in range(NK):
                        nc.tensor.matmul(psz2[:], w2all[:, k * DIM:(k + 1) * DIM],
                                         h1[:, 128 * k:128 * k + 128],
                                         start=(k == 0), stop=(k == NK - 1))
                    z2 = spool.tile([DIM, BC], f32, name='z2s', tag='z2s')
                    nc.scalar.activation(z2[:], psz2[:], AF.Identity,
                                         bias=b2, scale=s2[:, 0:1])
                    psE2 = pp1.tile([DIM, BC], f32, name='e2small', tag='epath')
                    for k in range(NK):
                        nc.tensor.matmul(psE2[:], w2e[:, k * DIM:(k + 1) * DIM],
                                         E1[:, 128 * k:128 * k + 128],
                                         start=(k == 0), stop=(k == NK - 1))
                    nc.vector.tensor_scalar(out=E2s[f][:], in0=psE2[:],
                                            scalar1=s2[:, 0:1], scalar2=None,
                                            op0=AO.mult)

                    # ---- gate mix / flip or final logp term
                    if f < 2:
                        sg, omsg = sgs[f]
                        ta = spool.tile([DIM, BC], f32, name='ta', tag='ta')
                        nc.gpsimd.tensor_scalar(out=ta[:], in0=z2[:], scalar1=sg[:],
                                                scalar2=None, op0=AO.mult)
                        tb = spool.tile([DIM, BC], f32, name='tb', tag='tb')
                        nc.gpsimd.tensor_scalar(out=tb[:], in0=xT[:], scalar1=omsg[:],
                                                scalar2=None, op0=AO.mult)
                        xmix = spool.tile([DIM, BC], bf16, name='xmix', tag='xmix')
                        nc.gpsimd.tensor_tensor(xmix[:], ta[:], tb[:], AO.add)
                        psf = pp.tile([DIM, BC], f32, name='zsmall', tag='zsmall',
                                      bufs=1)
                        nc.tensor.matmul(psf[:], flip64[:], xmix[:], start=True, stop=True)
                        nc.scalar.activation(xT[:], psf[:], AF.Copy)
                    else:
                        sqx = spool.tile([DIM, BC], f32, name='sqx', tag='sqx')
                        nc.scalar.square(sqx[:], z2[:])
                        nc.gpsimd.tensor_scalar(out=sqx[:], in0=sqx[:], scalar1=-0.5,
                                                scalar2=-0.5 * LOG_2PI, op0=AO.mult, op1=AO.add)
                        nc.gpsimd.tensor_tensor(acc[:], acc[:], sqx[:], AO.add)

              # ---- ldj tail (ln phase)
              for f in range(2):
                    u = spool.tile([DIM, BC], f32, name='u', tag='u')
                    nc.gpsimd.tensor_scalar(out=u[:], in0=E2s[f][:], scalar1=egs[f][:],
                                            scalar2=1.0, op0=AO.mult, op1=AO.add)
                    lf = spool.tile([DIM, BC], f32, name='lf', tag='lf')
                    nc.scalar.activation(lf[:], u[:], AF.Ln)
                    l1p = spool.tile([DIM, 1], f32, name='l1p', tag='l1p')
                    nc.vector.tensor_scalar(out=l1p[:], in0=egs[f][:], scalar1=1.0,
                                            scalar2=None, op0=AO.add)
                    nc.scalar.activation(l1p[:], l1p[:], AF.Ln)
                    nc.gpsimd.tensor_scalar(out=lf[:], in0=lf[:], scalar1=l1p[:],
                                            scalar2=None, op0=AO.subtract)
                    nc.gpsimd.tensor_tensor(acc[:], acc[:], lf[:], AO.add)
              lf2 = spool.tile([DIM, BC], f32, name='lf2', tag='lf2')
              nc.scalar.activation(lf2[:], E2s[2][:], AF.Ln)
              nc.gpsimd.tensor_tensor(acc[:], acc[:], lf2[:], AO.add)

              psum_out = pp.tile([1, BC], f32, name='zsmall', tag='zsmall',
                                 bufs=1)
              nc.tensor.matmul(psum_out[:], ones64[:], acc[:], start=True, stop=True)
              outs = spool.tile([1, BC], f32, name='outs', tag='outs')
              nc.vector.tensor_copy(outs[:], psum_out[:])
              nc.sync.dma_start(out=out[:], in_=outs[:])

    _split_sync_waits(nc)
    return nc


def _scol(nc, spool, AO, AF, wsn, dwc, shape, tag):
    """s = exp(dw - 0.5*ln(wsn)) as (P, ncols) tile."""
    import concourse.mybir as mybir
    f32 = mybir.dt.float32
    ln = spool.tile(list(shape), f32, name=tag + 'n' + 'ln')
    nc.scalar.activation(ln[:], wsn[:], AF.Ln)
    t = spool.tile(list(shape), f32, name=tag + 'n' + 't')
    nc.vector.tensor_scalar(out=t[:], in0=ln[:], scalar1=-0.5, scalar2=None, op0=AO.mult)
    nc.vector.tensor_tensor(t[:], t[:], dwc, AO.add)
    s = spool.tile(list(shape), f32, name=tag + 'n' + 's')
    nc.scalar.activation(s[:], t[:], AF.Exp)
    return s


# ------------------------------------------------------------------ runner
def _make_runner(nc, n_cores):
    import jax
    from jax.sharding import Mesh, PartitionSpec
    from jax.experimental.shard_map import shard_map
    import concourse.mybir as mybir
    from concourse.bass2jax import (_bass_exec_p, partition_id_tensor,
                                    install_neuronx_cc_hook)
    install_neuronx_cc_hook()
    partition_name = nc.partition_id_tensor.name if nc.partition_id_tensor else None
    in_names, out_names, out_avals = [], [], []
    for alloc in nc.m.functions[0].allocations:
        if not isinstance(alloc, mybir.MemoryLocationSet):
            continue
        name = alloc.memorylocations[0].name
        if alloc.kind == "ExternalInput":
            if name != partition_name:
                in_names.append(name)
        elif alloc.kind == "ExternalOutput":
            out_names.append(name)
            out_avals.append(jax.core.ShapedArray(
                tuple(alloc.tensor_shape), mybir.dt.np(alloc.dtype)))
    n_params = len(in_names)
    all_names = in_names + out_names + ([partition_name] if partition_name else [])

    def _body(*args):
        operands = list(args)
        if partition_name is not None:
            operands.append(partition_id_tensor())
        outs = _bass_exec_p.bind(
            *operands, out_avals=tuple(out_avals), in_names=tuple(all_names),
            out_names=tuple(out_names), lowering_input_output_aliases=(),
            sim_require_finite=False, sim_require_nnan=False, nc=nc)
        return tuple(outs)

    devices = jax.devices()[:n_cores]
    mesh = Mesh(np.asarray(devices), ("core",))
    n_outs = len(out_names)
    in_specs = (PartitionSpec("core"),) * (n_params + n_outs)
    out_specs = (PartitionSpec("core"),) * n_outs
    fn = jax.jit(shard_map(_body, mesh=mesh, in_specs=in_specs,
                           out_specs=out_specs, check_rep=False),
                 keep_unused=True)
    return fn, in_names, out_names, out_avals


def _get_runner():
    key = ('runner', MAIN_DT)
    if key not in _CACHE:
        import sys, os
        d = os.path.dirname(os.path.abspath(__file__))
        if d not in sys.path:
            sys.path.insert(0, d)
        nc = _build(MAIN_DT)
        _CACHE[key] = _make_runner(nc, NCORES)
    return _CACHE[key]


def kernel(**inputs):
    fl = _host_prep(inputs)
    x = np.asarray(inputs['x'])
    fn, in_names, out_names, out_avals = _get_runner()
    in_maps = []
    for c in range(NCORES):
        m = dict(fl)
        m['xT'] = np.ascontiguousarray(x[c * BC:(c + 1) * BC, :].T)
        in_maps.append(m)
    concat_in = [np.concatenate([np.asarray(m[name]) for m in in_maps], axis=0)
                 for name in in_names]
    concat_zeros = [np.zeros((NCORES * a.shape[0], *a.shape[1:]), a.dtype)
                    for a in out_avals]
    outs = fn(*concat_in, *concat_zeros)
    o = np.asarray(outs[0]).reshape(NCORES, BC)
    return o.reshape(B).astype(np.float32)
